# revision 21
# baseline (speedup 1.0000x reference)
"""Multi-head causal attention on 8 TRN2 NeuronCores — one head per core.

Full inputs in, full output out. Per core (head h):
  Q^T/K^T/V^T = W^T x^T   (PE, bf16)
  S^T[j,i] = K_j . Q_i    (PE, causal-packed, flash-style)
  P^T = exp(S^T/8)        (ScalarE, no max-subtraction: |scores| << 1)
  O^T[v,i] accum += V'[j,(v|1)]^T P^T[j,i]  (PE; row 64 = sumexp)
  out[i,o] = (O^T[:,i]/sumexp_i)^T W_o      (PE + fused row scale on evac)
Host sums the 8 per-head partial outputs.
"""

import numpy as np
import ml_dtypes

import concourse.bass as bass
import concourse.mybir as mybir
import concourse.tile as tile
from concourse import bacc
from concourse.bass_utils import run_bass_kernel_spmd

BF16 = mybir.dt.bfloat16
F32 = mybir.dt.float32

S = 4096
D_IN = 512
D_K = 64
D_V = 64
D_OUT = 512
H = 8
NJT = S // 128   # 32 key tiles
NCH = S // 512   # 8 query chunks
NCK = D_IN // 128  # 4 contraction chunks for projections

_CACHE = {}


def _emit(nc, tc, ctx_pools):
    import contextlib

    xT_d = nc.dram_tensor("xT", [D_IN, S], BF16, kind="ExternalInput").ap()
    wq_d = nc.dram_tensor("wq", [D_IN, 128], BF16, kind="ExternalInput").ap()
    wk_d = nc.dram_tensor("wk", [D_IN, 128], BF16, kind="ExternalInput").ap()
    wv_d = nc.dram_tensor("wv", [D_IN, D_V], BF16, kind="ExternalInput").ap()
    wo_d = nc.dram_tensor("wo", [D_V, D_OUT], BF16, kind="ExternalInput").ap()
    mask_d = nc.dram_tensor("mask", [128, 128], BF16, kind="ExternalInput").ap()
    iden_d = nc.dram_tensor("iden", [128, 128], BF16, kind="ExternalInput").ap()
    out_d = nc.dram_tensor("out", [S, D_OUT], F32, kind="ExternalOutput").ap()

    Exp = mybir.ActivationFunctionType.Exp

    with contextlib.ExitStack() as ctx:
        const = ctx.enter_context(tc.tile_pool(name="const", bufs=1))
        persist = ctx.enter_context(tc.tile_pool(name="persist", bufs=1))
        small = ctx.enter_context(tc.tile_pool(name="small", bufs=4))
        outp = ctx.enter_context(tc.tile_pool(name="outp", bufs=4))

        # ---- constants ----
        # wq/wk arrive column-duplicated [512, 128] so the projection fills
        # both partition halves (enables PE row tiles T0+T8 in pass 1)
        wq_sb = const.tile([128, NCK * 128], BF16)
        wk_sb = const.tile([128, NCK * 128], BF16)
        wv_sb = const.tile([128, NCK * D_V], BF16)
        wo_sb = const.tile([D_V, D_OUT], BF16)
        mask_sb = const.tile([128, 128], BF16)
        iden_sb = const.tile([128, 128], BF16)
        for c in range(NCK):
            rows = slice(c * 128, (c + 1) * 128)
            nc.sync.dma_start(out=wq_sb[:, c * 128:(c + 1) * 128], in_=wq_d[rows, :])
            nc.sync.dma_start(out=wk_sb[:, c * 128:(c + 1) * 128], in_=wk_d[rows, :])
            nc.sync.dma_start(out=wv_sb[:, c * D_V:(c + 1) * D_V], in_=wv_d[rows, :])
        nc.sync.dma_start(out=wo_sb, in_=wo_d)
        nc.sync.dma_start(out=mask_sb, in_=mask_d)
        nc.sync.dma_start(out=iden_sb, in_=iden_d)

        # persistent activations
        qt = persist.tile([128, S], BF16)   # Q^T duplicated in both halves
        kt = persist.tile([128, S], BF16)   # K^T duplicated in both halves
        vpt = persist.tile([64, S], BF16)   # V^T
        vp = persist.tile([128, NJT * 65], BF16)  # V' tiles [128, 65] per jt

        # ones column of every V' tile: strided [128, NJT] memset
        nc.vector.memset(
            vp.rearrange("p (j w) -> p j w", w=65)[:, :, 64], 1.0)

        # ---- stage A: projections Q^T/K^T/V^T = W^T x^T ----
        with tc.tile_pool(name="xt", bufs=1) as xtp, \
             tc.tile_pool(name="psA", bufs=4, space="PSUM") as psA:
            xts = []
            for c in range(NCK):
                xt = xtp.tile([128, S], BF16, tag=f"xt{c}")
                nc.sync.dma_start(out=xt, in_=xT_d[c * 128:(c + 1) * 128, :])
                xts.append(xt)
            for st in range(NCH):
                sl = bass.ts(st, 512)
                for w_sb, wid, dest, dcopy in (
                    (wq_sb, 128, qt, nc.vector.tensor_copy),
                    (wk_sb, 128, kt, nc.scalar.copy),
                    (wv_sb, 64, vpt, nc.vector.tensor_copy),
                ):
                    ps = psA.tile([wid, 512], F32, tag="psA",
                                  name=f"psA{st}_{wid}")
                    for c in range(NCK):
                        nc.tensor.matmul(
                            ps,
                            lhsT=w_sb[:, c * wid:(c + 1) * wid],
                            rhs=xts[c][:, sl],
                            start=(c == 0),
                            stop=(c == NCK - 1),
                        )
                    dcopy(dest[:, sl], ps)



        # ---- fused pass: S^T+exp, with O^T bursts filling PE exp-wait gaps ----
        from collections import deque
        pt_pool = ctx.enter_context(tc.tile_pool(name="pt", bufs=1))
        pts = []
        pending = deque()  # closures, each emits one PE-side step of pass 2

        def drain(n):
            if len(pending) > 48:
                n += 4
            for _ in range(n):
                if not pending:
                    return
                pending.popleft()()

        def vp_transpose(jt):
            def go():
                pst = psAcc.tile([128, 64], BF16, tag="bank", name=f"pst{jt}")
                nc.tensor.transpose(
                    pst,
                    vpt[:, jt * 128:(jt + 1) * 128],
                    iden_sb[0:64, 0:64],
                )
                nc.vector.tensor_copy(vp[:, jt * 65:jt * 65 + 64], pst)
            return go

        def enqueue_chunk(c, jt_last):
            acc = psAcc.tile([65, 512], F32, tag="bank", name=f"acc{c}")

            def ot_mm(j2):
                def go():
                    lo = max(c * 512, j2 * 128)
                    hi = (c + 1) * 512
                    nc.tensor.matmul(
                        acc[:, lo - c * 512:hi - c * 512],
                        lhsT=vp[:, j2 * 65:(j2 + 1) * 65],
                        rhs=pts[j2][:, lo - j2 * 128:hi - j2 * 128],
                        start=(j2 == 0),
                        stop=(j2 == jt_last),
                    )
                return go

            for j2 in range(jt_last + 1):
                pending.append(ot_mm(j2))

            def evac():
                ot_bf = small.tile([65, 512], BF16, tag="otbf")
                nc.vector.tensor_copy(ot_bf, acc)
                se_bf = small.tile([128, 4], BF16, tag="se_bf")
                for ib in range(4):
                    nc.gpsimd.dma_start(
                        out=se_bf[:, ib:ib + 1],
                        in_=ot_bf[64:65, ib * 128:(ib + 1) * 128],
                    )
                rcols = small.tile([128, 4], F32, tag="rcols")
                nc.vector.reciprocal(rcols, se_bf)

                def out_proj(ib):
                    def go():
                        po = psAcc.tile([128, 512], F32, tag="bank",
                                        name=f"po{c}_{ib}")
                        nc.tensor.matmul(
                            po,
                            lhsT=ot_bf[0:64, ib * 128:(ib + 1) * 128],
                            rhs=wo_sb,
                            start=True,
                            stop=True,
                        )
                        ob = outp.tile([128, 512], F32, tag="ob")
                        nc.vector.tensor_scalar_mul(ob, po, rcols[:, ib:ib + 1])
                        nc.sync.dma_start(
                            out=out_d[c * 512 + ib * 128:
                                      c * 512 + (ib + 1) * 128, :],
                            in_=ob,
                        )
                    return go

                for ib in range(4):
                    pending.append(out_proj(ib))

            pending.append(evac)

        with tc.tile_pool(name="psB", bufs=1, space="PSUM") as psB, \
             tc.tile_pool(name="psAcc", bufs=4, space="PSUM") as psAcc:
            for jt in range(NJT):
                pending.append(vp_transpose(jt))
            for jt in range(NJT):
                i0 = jt * 128           # diagonal start
                c0 = jt // 4            # first query chunk
                pt = pt_pool.tile([128, S - i0], BF16, tag=f"pt{jt}")
                pts.append(pt)
                pb = 64 * (jt % 2)  # alternate PE row tiles T0/T8
                ktile = kt[pb:pb + 64, jt * 128:(jt + 1) * 128]
                for g0 in range(c0, NCH, 4):
                    g1 = min(g0 + 4, NCH)
                    ps = psB.tile([128, 2048], F32, tag="psB")
                    for c in range(g0, g1):
                        lo = max(c * 512, i0)
                        hi = (c + 1) * 512
                        nc.tensor.matmul(
                            ps[:, (c - g0) * 512 + lo - c * 512:
                                  (c - g0) * 512 + hi - c * 512],
                            lhsT=ktile,
                            rhs=qt[pb:pb + 64, lo:hi],
                            start=True,
                            stop=True,
                        )
                    glo = max(g0 * 512, i0)
                    ghi = g1 * 512
                    nc.scalar.activation(
                        pt[:, glo - i0:ghi - i0],
                        ps[:, glo - g0 * 512:ghi - g0 * 512],
                        Exp,
                        scale=0.125,
                    )
                    drain(4)  # O^T/out-proj work while ScalarE runs exp
                # causal mask on the diagonal 128x128 block
                nc.vector.tensor_mul(pt[:, 0:128], pt[:, 0:128], mask_sb)
                if jt % 4 == 3:
                    enqueue_chunk(jt // 4, jt)
            while pending:
                drain(8)


def _build():
    if "nc" in _CACHE:
        return _CACHE["nc"]
    nc = bacc.Bacc("TRN2", target_bir_lowering=False, debug=False)
    with tile.TileContext(nc) as tc:
        _emit(nc, tc, None)
    nc.compile()
    _CACHE["nc"] = nc
    return nc


def build_in_maps(x, W_q, W_k, W_v, W_o):
    bf = ml_dtypes.bfloat16
    xT = np.ascontiguousarray(x.reshape(S, D_IN).T).astype(bf)
    mask = np.triu(np.ones((128, 128), np.float32)).astype(bf)
    iden = np.eye(128, dtype=np.float32).astype(bf)
    in_maps = []
    for h in range(H):
        wq2 = np.concatenate([W_q[h], W_q[h]], axis=1)  # [512, 128]
        wk2 = np.concatenate([W_k[h], W_k[h]], axis=1)
        in_maps.append({
            "xT": xT,
            "wq": np.ascontiguousarray(wq2).astype(bf),
            "wk": np.ascontiguousarray(wk2).astype(bf),
            "wv": np.ascontiguousarray(W_v[h]).astype(bf),
            "wo": np.ascontiguousarray(W_o[h]).astype(bf),
            "mask": mask,
            "iden": iden,
        })
    return in_maps


def kernel(x, W_q, W_k, W_v, W_o):
    nc = _build()
    in_maps = build_in_maps(x, W_q, W_k, W_v, W_o)
    res = run_bass_kernel_spmd(nc, in_maps, core_ids=list(range(H)))
    out = np.zeros((S, D_OUT), np.float32)
    for h in range(H):
        out += res.results[h]["out"]
    return out[None]


# revision 30
# speedup vs baseline: 1.3243x; 1.3243x over previous
"""Multi-head causal attention on 8 TRN2 NeuronCores — one head per core.

Full inputs in, full output out. Per core (head h):
  Q^T/K^T/V^T = W^T x^T   (PE, bf16)
  S^T[j,i] = K_j . Q_i    (PE, causal-packed, flash-style)
  P^T = exp(S^T/8)        (ScalarE, no max-subtraction: |scores| << 1)
  O^T[v,i] accum += V'[j,(v|1)]^T P^T[j,i]  (PE; row 64 = sumexp)
  out[i,o] = (O^T[:,i]/sumexp_i)^T W_o      (PE + fused row scale on evac)
Host sums the 8 per-head partial outputs.
"""

import numpy as np
import ml_dtypes

import concourse.bass as bass
import concourse.mybir as mybir
import concourse.tile as tile
from concourse import bacc
from concourse.bass_utils import run_bass_kernel_spmd

BF16 = mybir.dt.bfloat16
F32 = mybir.dt.float32

S = 4096
D_IN = 512
D_K = 64
D_V = 64
D_OUT = 512
H = 8
NJT = S // 128   # 32 key tiles
NCH = S // 512   # 8 query chunks
NCK = D_IN // 128  # 4 contraction chunks for projections

_CACHE = {}


def _emit(nc, tc, ctx_pools):
    import contextlib

    xT_d = nc.dram_tensor("xT", [D_IN, S], BF16, kind="ExternalInput").ap()
    wq_d = nc.dram_tensor("wq", [D_IN, 128], BF16, kind="ExternalInput").ap()
    wk_d = nc.dram_tensor("wk", [D_IN, 128], BF16, kind="ExternalInput").ap()
    wv_d = nc.dram_tensor("wv", [D_IN, D_V], BF16, kind="ExternalInput").ap()
    wo_d = nc.dram_tensor("wo", [D_V, D_OUT], BF16, kind="ExternalInput").ap()
    mask_d = nc.dram_tensor("mask", [128, 128], BF16, kind="ExternalInput").ap()
    iden_d = nc.dram_tensor("iden", [128, 128], BF16, kind="ExternalInput").ap()
    out_d = nc.dram_tensor("out", [S, D_OUT], F32, kind="ExternalOutput").ap()

    Exp = mybir.ActivationFunctionType.Exp

    with contextlib.ExitStack() as ctx:
        const = ctx.enter_context(tc.tile_pool(name="const", bufs=1))
        persist = ctx.enter_context(tc.tile_pool(name="persist", bufs=1))
        small = ctx.enter_context(tc.tile_pool(name="small", bufs=4))
        outp = ctx.enter_context(tc.tile_pool(name="outp", bufs=4))

        # ---- constants ----
        # wq/wk arrive column-duplicated [512, 128] so the projection fills
        # both partition halves (enables PE row tiles T0+T8 in pass 1)
        wq_sb = const.tile([128, NCK * 128], BF16)
        wk_sb = const.tile([128, NCK * 128], BF16)
        wv_sb = const.tile([128, NCK * D_V], BF16)
        wo_sb = const.tile([D_V, D_OUT], BF16)
        mask_sb = const.tile([128, 128], BF16)
        iden_sb = const.tile([128, 128], BF16)
        for c in range(NCK):
            rows = slice(c * 128, (c + 1) * 128)
            nc.sync.dma_start(out=wq_sb[:, c * 128:(c + 1) * 128], in_=wq_d[rows, :])
            nc.sync.dma_start(out=wk_sb[:, c * 128:(c + 1) * 128], in_=wk_d[rows, :])
            nc.sync.dma_start(out=wv_sb[:, c * D_V:(c + 1) * D_V], in_=wv_d[rows, :])
        nc.sync.dma_start(out=wo_sb, in_=wo_d)
        nc.sync.dma_start(out=mask_sb, in_=mask_d)
        nc.sync.dma_start(out=iden_sb, in_=iden_d)

        # persistent activations
        qt = persist.tile([128, S], BF16)   # Q^T duplicated in both halves
        kt = persist.tile([128, S], BF16)   # K^T duplicated in both halves
        vpt = persist.tile([64, S], BF16)   # V^T
        vp = persist.tile([128, NJT * 65], BF16)  # V' tiles [128, 65] per jt

        # ones column of every V' tile: strided [128, NJT] memset
        nc.vector.memset(
            vp.rearrange("p (j w) -> p j w", w=65)[:, :, 64], 1.0)

        # ---- stage A: projections Q^T/K^T/V^T = W^T x^T ----
        with tc.tile_pool(name="xt", bufs=1) as xtp, \
             tc.tile_pool(name="psA", bufs=4, space="PSUM") as psA:
            xts = []
            for c in range(NCK):
                xt = xtp.tile([128, S], BF16, tag=f"xt{c}")
                nc.sync.dma_start(out=xt[:, :S // 2],
                                  in_=xT_d[c * 128:(c + 1) * 128, :S // 2])
                xts.append(xt)
            for c in range(NCK):
                nc.sync.dma_start(out=xts[c][:, S // 2:],
                                  in_=xT_d[c * 128:(c + 1) * 128, S // 2:])
            for w_sb, wid, dest, dcopy in (
                (wq_sb, 128, qt, nc.vector.tensor_copy),
                (wk_sb, 128, kt, nc.scalar.copy),
                (wv_sb, 64, vpt, nc.scalar.copy),
            ):
                for st in range(NCH):
                    sl = bass.ts(st, 512)
                    ps = psA.tile([wid, 512], F32, tag="psA",
                                  name=f"psA{st}_{wid}")
                    for c in range(NCK):
                        nc.tensor.matmul(
                            ps,
                            lhsT=w_sb[:, c * wid:(c + 1) * wid],
                            rhs=xts[c][:, sl],
                            start=(c == 0),
                            stop=(c == NCK - 1),
                        )
                    dcopy(dest[:, sl], ps)



        # ---- fused pass: S^T+exp, with O^T bursts filling PE exp-wait gaps ----
        from collections import deque
        pt_pool = ctx.enter_context(tc.tile_pool(name="pt", bufs=1))
        pts = []
        pending = deque()  # closures, each emits one PE-side step of pass 2

        def drain(n):
            if len(pending) > 48:
                n += 4
            for _ in range(n):
                if not pending:
                    return
                pending.popleft()()

        def vp_transpose(jt):
            def go():
                pst = psAcc.tile([128, 64], BF16, tag="bank", name=f"pst{jt}")
                nc.tensor.transpose(
                    pst,
                    vpt[:, jt * 128:(jt + 1) * 128],
                    iden_sb[0:64, 0:64],
                )
                nc.vector.tensor_copy(vp[:, jt * 65:jt * 65 + 64], pst)
            return go

        def enqueue_chunk(c, jt_last):
            acc = psAcc.tile([65, 512], F32, tag="bank", name=f"acc{c}")

            def ot_mm(j2):
                def go():
                    lo = max(c * 512, j2 * 128)
                    hi = (c + 1) * 512
                    nc.tensor.matmul(
                        acc[:, lo - c * 512:hi - c * 512],
                        lhsT=vp[:, j2 * 65:(j2 + 1) * 65],
                        rhs=pts[j2][:, lo - j2 * 128:hi - j2 * 128],
                        start=(j2 == 0),
                        stop=(j2 == jt_last),
                    )
                return go

            for j2 in range(jt_last + 1):
                pending.append(ot_mm(j2))

            def evac():
                ot_bf = small.tile([65, 512], BF16, tag="otbf")
                nc.vector.tensor_copy(ot_bf, acc)
                se_bf = small.tile([128, 4], BF16, tag="se_bf")
                for ib in range(4):
                    nc.gpsimd.dma_start(
                        out=se_bf[:, ib:ib + 1],
                        in_=ot_bf[64:65, ib * 128:(ib + 1) * 128],
                    )
                rcols = small.tile([128, 4], F32, tag="rcols")
                nc.vector.reciprocal(rcols, se_bf)

                def out_proj(ib):
                    def go():
                        po = psAcc.tile([128, 512], F32, tag="bank",
                                        name=f"po{c}_{ib}")
                        nc.tensor.matmul(
                            po,
                            lhsT=ot_bf[0:64, ib * 128:(ib + 1) * 128],
                            rhs=wo_sb,
                            start=True,
                            stop=True,
                        )
                        ob = outp.tile([128, 512], F32, tag="ob")
                        nc.vector.tensor_scalar_mul(ob, po, rcols[:, ib:ib + 1])
                        nc.sync.dma_start(
                            out=out_d[c * 512 + ib * 128:
                                      c * 512 + (ib + 1) * 128, :],
                            in_=ob,
                        )
                    return go

                for ib in range(4):
                    pending.append(out_proj(ib))

            pending.append(evac)

        with tc.tile_pool(name="psB", bufs=2, space="PSUM") as psB, \
             tc.tile_pool(name="psAcc", bufs=4, space="PSUM") as psAcc:
            for jt in range(NJT):
                pending.append(vp_transpose(jt))
            for jt in range(NJT):
                i0 = jt * 128           # diagonal start
                c0 = jt // 4            # first query chunk
                pt = pt_pool.tile([128, S - i0], BF16, tag=f"pt{jt}")
                pts.append(pt)
                # full 128-row contraction over the duplicated halves: each
                # product is summed twice (folded into exp scale), which keeps
                # the PE activity monitor warm (2.4 GHz) vs 64-row matmuls
                ktile = kt[:, jt * 128:(jt + 1) * 128]
                for g0 in range(c0, NCH, 2):
                    g1 = min(g0 + 2, NCH)
                    ps = psB.tile([128, 1024], F32, tag="psB")
                    for c in range(g0, g1):
                        lo = max(c * 512, i0)
                        hi = (c + 1) * 512
                        nc.tensor.matmul(
                            ps[:, (c - g0) * 512 + lo - c * 512:
                                  (c - g0) * 512 + hi - c * 512],
                            lhsT=ktile,
                            rhs=qt[:, lo:hi],
                            start=True,
                            stop=True,
                        )
                    glo = max(g0 * 512, i0)
                    ghi = g1 * 512
                    nc.scalar.activation(
                        pt[:, glo - i0:ghi - i0],
                        ps[:, glo - g0 * 512:ghi - g0 * 512],
                        Exp,
                        scale=0.0625,  # 1/sqrt(64) / 2 (duplicated contraction)
                    )
                    drain(3)  # O^T/out-proj work while ScalarE runs exp
                # causal mask on the diagonal 128x128 block
                nc.vector.tensor_mul(pt[:, 0:128], pt[:, 0:128], mask_sb)
                if jt % 4 == 3:
                    enqueue_chunk(jt // 4, jt)
            while pending:
                drain(8)


def _build():
    if "nc" in _CACHE:
        return _CACHE["nc"]
    nc = bacc.Bacc("TRN2", target_bir_lowering=False, debug=False)
    with tile.TileContext(nc) as tc:
        _emit(nc, tc, None)
    nc.compile()
    _CACHE["nc"] = nc
    return nc


def build_in_maps(x, W_q, W_k, W_v, W_o):
    bf = ml_dtypes.bfloat16
    xT = np.ascontiguousarray(x.reshape(S, D_IN).T).astype(bf)
    mask = np.triu(np.ones((128, 128), np.float32)).astype(bf)
    iden = np.eye(128, dtype=np.float32).astype(bf)
    in_maps = []
    for h in range(H):
        wq2 = np.concatenate([W_q[h], W_q[h]], axis=1)  # [512, 128]
        wk2 = np.concatenate([W_k[h], W_k[h]], axis=1)
        in_maps.append({
            "xT": xT,
            "wq": np.ascontiguousarray(wq2).astype(bf),
            "wk": np.ascontiguousarray(wk2).astype(bf),
            "wv": np.ascontiguousarray(W_v[h]).astype(bf),
            "wo": np.ascontiguousarray(W_o[h]).astype(bf),
            "mask": mask,
            "iden": iden,
        })
    return in_maps


def kernel(x, W_q, W_k, W_v, W_o):
    nc = _build()
    in_maps = build_in_maps(x, W_q, W_k, W_v, W_o)
    res = run_bass_kernel_spmd(nc, in_maps, core_ids=list(range(H)))
    out = np.zeros((S, D_OUT), np.float32)
    for h in range(H):
        out += res.results[h]["out"]
    return out[None]


# revision 32
# speedup vs baseline: 1.3696x; 1.0342x over previous
"""Multi-head causal attention on 8 TRN2 NeuronCores — one head per core.

Full inputs in, full output out. Per core (head h):
  Q^T/K^T/V^T = W^T x^T   (PE, bf16)
  S^T[j,i] = K_j . Q_i    (PE, causal-packed, flash-style)
  P^T = exp(S^T/8)        (ScalarE, no max-subtraction: |scores| << 1)
  O^T[v,i] accum += V'[j,(v|1)]^T P^T[j,i]  (PE; row 64 = sumexp)
  out[i,o] = (O^T[:,i]/sumexp_i)^T W_o      (PE + fused row scale on evac)
Host sums the 8 per-head partial outputs.
"""

import numpy as np
import ml_dtypes

import concourse.bass as bass
import concourse.mybir as mybir
import concourse.tile as tile
from concourse import bacc
from concourse.bass_utils import run_bass_kernel_spmd

BF16 = mybir.dt.bfloat16
F32 = mybir.dt.float32

S = 4096
D_IN = 512
D_K = 64
D_V = 64
D_OUT = 512
H = 8
NJT = S // 128   # 32 key tiles
NCH = S // 512   # 8 query chunks
NCK = D_IN // 128  # 4 contraction chunks for projections

_CACHE = {}


def _emit(nc, tc, ctx_pools):
    import contextlib

    xT_d = nc.dram_tensor("xT", [D_IN, S], BF16, kind="ExternalInput").ap()
    wq_d = nc.dram_tensor("wq", [D_IN, 128], BF16, kind="ExternalInput").ap()
    wk_d = nc.dram_tensor("wk", [D_IN, 128], BF16, kind="ExternalInput").ap()
    wv_d = nc.dram_tensor("wv", [D_IN, D_V], BF16, kind="ExternalInput").ap()
    wo_d = nc.dram_tensor("wo", [D_V, D_OUT], BF16, kind="ExternalInput").ap()
    mask_d = nc.dram_tensor("mask", [128, 128], BF16, kind="ExternalInput").ap()
    iden_d = nc.dram_tensor("iden", [128, 128], BF16, kind="ExternalInput").ap()
    out_d = nc.dram_tensor("out", [S, D_OUT], F32, kind="ExternalOutput").ap()

    Exp = mybir.ActivationFunctionType.Exp

    with contextlib.ExitStack() as ctx:
        const = ctx.enter_context(tc.tile_pool(name="const", bufs=1))
        persist = ctx.enter_context(tc.tile_pool(name="persist", bufs=1))
        small = ctx.enter_context(tc.tile_pool(name="small", bufs=4))
        outp = ctx.enter_context(tc.tile_pool(name="outp", bufs=4))

        # ---- constants ----
        # wq/wk arrive column-duplicated [512, 128] so the projection fills
        # both partition halves (enables PE row tiles T0+T8 in pass 1)
        wq_sb = const.tile([128, NCK * 128], BF16)
        wk_sb = const.tile([128, NCK * 128], BF16)
        wv_sb = const.tile([128, NCK * D_V], BF16)
        wo_sb = const.tile([D_V, D_OUT], BF16)
        mask_sb = const.tile([128, 128], BF16)
        iden_sb = const.tile([128, 128], BF16)
        for c in range(NCK):
            rows = slice(c * 128, (c + 1) * 128)
            nc.gpsimd.dma_start(out=wq_sb[:, c * 128:(c + 1) * 128], in_=wq_d[rows, :])
            nc.gpsimd.dma_start(out=wk_sb[:, c * 128:(c + 1) * 128], in_=wk_d[rows, :])
            nc.gpsimd.dma_start(out=wv_sb[:, c * D_V:(c + 1) * D_V], in_=wv_d[rows, :])
        nc.gpsimd.dma_start(out=wo_sb, in_=wo_d)
        nc.gpsimd.dma_start(out=mask_sb, in_=mask_d)
        nc.gpsimd.dma_start(out=iden_sb, in_=iden_d)

        # persistent activations
        qt = persist.tile([128, S], BF16)   # Q^T duplicated in both halves
        kt = persist.tile([128, S], BF16)   # K^T duplicated in both halves
        vpt = persist.tile([64, S], BF16)   # V^T
        vp = persist.tile([128, NJT * 65], BF16)  # V' tiles [128, 65] per jt

        # ones column of every V' tile: strided [128, NJT] memset
        nc.vector.memset(
            vp.rearrange("p (j w) -> p j w", w=65)[:, :, 64], 1.0)

        # ---- stage A: projections Q^T/K^T/V^T = W^T x^T ----
        with tc.tile_pool(name="xt", bufs=1) as xtp, \
             tc.tile_pool(name="psA", bufs=4, space="PSUM") as psA:
            xts = []
            for c in range(NCK):
                xt = xtp.tile([128, S], BF16, tag=f"xt{c}")
                nc.sync.dma_start(out=xt, in_=xT_d[c * 128:(c + 1) * 128, :])
                xts.append(xt)
            for w_sb, wid, dest, dcopy in (
                (wq_sb, 128, qt, nc.vector.tensor_copy),
                (wk_sb, 128, kt, nc.scalar.copy),
                (wv_sb, 64, vpt, nc.scalar.copy),
            ):
                for st in range(NCH):
                    sl = bass.ts(st, 512)
                    ps = psA.tile([wid, 512], F32, tag="psA",
                                  name=f"psA{st}_{wid}")
                    for c in range(NCK):
                        nc.tensor.matmul(
                            ps,
                            lhsT=w_sb[:, c * wid:(c + 1) * wid],
                            rhs=xts[c][:, sl],
                            start=(c == 0),
                            stop=(c == NCK - 1),
                        )
                    dcopy(dest[:, sl], ps)



        # ---- fused pass: S^T+exp, with O^T bursts filling PE exp-wait gaps ----
        from collections import deque
        pt_pool = ctx.enter_context(tc.tile_pool(name="pt", bufs=1))
        pts = []
        pending = deque()  # closures, each emits one PE-side step of pass 2

        def drain(n):
            if len(pending) > 48:
                n += 4
            for _ in range(n):
                if not pending:
                    return
                pending.popleft()()

        def vp_transpose(jt):
            def go():
                pst = psAcc.tile([128, 64], BF16, tag="bank", name=f"pst{jt}")
                nc.tensor.transpose(
                    pst,
                    vpt[:, jt * 128:(jt + 1) * 128],
                    iden_sb[0:64, 0:64],
                )
                nc.vector.tensor_copy(vp[:, jt * 65:jt * 65 + 64], pst)
            return go

        def enqueue_chunk(c, jt_last):
            acc = psAcc.tile([65, 512], F32, tag="bank", name=f"acc{c}")

            def ot_mm(j2):
                def go():
                    lo = max(c * 512, j2 * 128)
                    hi = (c + 1) * 512
                    nc.tensor.matmul(
                        acc[:, lo - c * 512:hi - c * 512],
                        lhsT=vp[:, j2 * 65:(j2 + 1) * 65],
                        rhs=pts[j2][:, lo - j2 * 128:hi - j2 * 128],
                        start=(j2 == 0),
                        stop=(j2 == jt_last),
                    )
                return go

            for j2 in range(jt_last + 1):
                pending.append(ot_mm(j2))

            def evac():
                ot_bf = small.tile([65, 512], BF16, tag="otbf")
                nc.vector.tensor_copy(ot_bf, acc)
                se_bf = small.tile([128, 4], BF16, tag="se_bf")
                for ib in range(4):
                    nc.gpsimd.dma_start(
                        out=se_bf[:, ib:ib + 1],
                        in_=ot_bf[64:65, ib * 128:(ib + 1) * 128],
                    )
                rcols = small.tile([128, 4], F32, tag="rcols")
                nc.vector.reciprocal(rcols, se_bf)

                def out_proj(ib):
                    def go():
                        po = psAcc.tile([128, 512], F32, tag="bank",
                                        name=f"po{c}_{ib}")
                        nc.tensor.matmul(
                            po,
                            lhsT=ot_bf[0:64, ib * 128:(ib + 1) * 128],
                            rhs=wo_sb,
                            start=True,
                            stop=True,
                        )
                        ob = outp.tile([128, 512], F32, tag="ob")
                        nc.vector.tensor_scalar_mul(ob, po, rcols[:, ib:ib + 1])
                        nc.sync.dma_start(
                            out=out_d[c * 512 + ib * 128:
                                      c * 512 + (ib + 1) * 128, :],
                            in_=ob,
                        )
                    return go

                for ib in range(4):
                    pending.append(out_proj(ib))

            pending.append(evac)

        with tc.tile_pool(name="psB", bufs=2, space="PSUM") as psB, \
             tc.tile_pool(name="psAcc", bufs=4, space="PSUM") as psAcc:
            for jt in range(NJT):
                pending.append(vp_transpose(jt))
            for jt in range(NJT):
                i0 = jt * 128           # diagonal start
                c0 = jt // 4            # first query chunk
                pt = pt_pool.tile([128, S - i0], BF16, tag=f"pt{jt}")
                pts.append(pt)
                # full 128-row contraction over the duplicated halves: each
                # product is summed twice (folded into exp scale), which keeps
                # the PE activity monitor warm (2.4 GHz) vs 64-row matmuls
                ktile = kt[:, jt * 128:(jt + 1) * 128]
                for g0 in range(c0, NCH, 2):
                    g1 = min(g0 + 2, NCH)
                    ps = psB.tile([128, 1024], F32, tag="psB")
                    for c in range(g0, g1):
                        lo = max(c * 512, i0)
                        hi = (c + 1) * 512
                        nc.tensor.matmul(
                            ps[:, (c - g0) * 512 + lo - c * 512:
                                  (c - g0) * 512 + hi - c * 512],
                            lhsT=ktile,
                            rhs=qt[:, lo:hi],
                            start=True,
                            stop=True,
                        )
                    glo = max(g0 * 512, i0)
                    ghi = g1 * 512
                    nc.scalar.activation(
                        pt[:, glo - i0:ghi - i0],
                        ps[:, glo - g0 * 512:ghi - g0 * 512],
                        Exp,
                        scale=0.0625,  # 1/sqrt(64) / 2 (duplicated contraction)
                    )
                    drain(3)  # O^T/out-proj work while ScalarE runs exp
                # causal mask on the diagonal 128x128 block
                nc.vector.tensor_mul(pt[:, 0:128], pt[:, 0:128], mask_sb)
                if jt % 4 == 3:
                    enqueue_chunk(jt // 4, jt)
            while pending:
                drain(8)


def _build():
    if "nc" in _CACHE:
        return _CACHE["nc"]
    nc = bacc.Bacc("TRN2", target_bir_lowering=False, debug=False)
    with tile.TileContext(nc) as tc:
        _emit(nc, tc, None)
    nc.compile()
    _CACHE["nc"] = nc
    return nc


def build_in_maps(x, W_q, W_k, W_v, W_o):
    bf = ml_dtypes.bfloat16
    xT = np.ascontiguousarray(x.reshape(S, D_IN).T).astype(bf)
    mask = np.triu(np.ones((128, 128), np.float32)).astype(bf)
    iden = np.eye(128, dtype=np.float32).astype(bf)
    in_maps = []
    for h in range(H):
        wq2 = np.concatenate([W_q[h], W_q[h]], axis=1)  # [512, 128]
        wk2 = np.concatenate([W_k[h], W_k[h]], axis=1)
        in_maps.append({
            "xT": xT,
            "wq": np.ascontiguousarray(wq2).astype(bf),
            "wk": np.ascontiguousarray(wk2).astype(bf),
            "wv": np.ascontiguousarray(W_v[h]).astype(bf),
            "wo": np.ascontiguousarray(W_o[h]).astype(bf),
            "mask": mask,
            "iden": iden,
        })
    return in_maps


def kernel(x, W_q, W_k, W_v, W_o):
    nc = _build()
    in_maps = build_in_maps(x, W_q, W_k, W_v, W_o)
    res = run_bass_kernel_spmd(nc, in_maps, core_ids=list(range(H)))
    out = np.zeros((S, D_OUT), np.float32)
    for h in range(H):
        out += res.results[h]["out"]
    return out[None]


# revision 35
# speedup vs baseline: 1.3884x; 1.0137x over previous
"""Multi-head causal attention on 8 TRN2 NeuronCores — one head per core.

Full inputs in, full output out. Per core (head h):
  Q^T/K^T/V^T = W^T x^T   (PE, bf16)
  S^T[j,i] = K_j . Q_i    (PE, causal-packed, flash-style)
  P^T = exp(S^T/8)        (ScalarE, no max-subtraction: |scores| << 1)
  O^T[v,i] accum += V'[j,(v|1)]^T P^T[j,i]  (PE; row 64 = sumexp)
  out[i,o] = (O^T[:,i]/sumexp_i)^T W_o      (PE + fused row scale on evac)
Host sums the 8 per-head partial outputs.
"""

import numpy as np
import ml_dtypes

import concourse.bass as bass
import concourse.mybir as mybir
import concourse.tile as tile
from concourse import bacc
from concourse.bass_utils import run_bass_kernel_spmd

BF16 = mybir.dt.bfloat16
F32 = mybir.dt.float32

S = 4096
D_IN = 512
D_K = 64
D_V = 64
D_OUT = 512
H = 8
NJT = S // 128   # 32 key tiles
NCH = S // 512   # 8 query chunks
NCK = D_IN // 128  # 4 contraction chunks for projections

_CACHE = {}


def _emit(nc, tc, ctx_pools):
    import contextlib

    xT_d = nc.dram_tensor("xT", [D_IN, S], BF16, kind="ExternalInput").ap()
    wq_d = nc.dram_tensor("wq", [D_IN, 128], BF16, kind="ExternalInput").ap()
    wk_d = nc.dram_tensor("wk", [D_IN, 128], BF16, kind="ExternalInput").ap()
    wv_d = nc.dram_tensor("wv", [D_IN, D_V], BF16, kind="ExternalInput").ap()
    wo_d = nc.dram_tensor("wo", [D_V, D_OUT], BF16, kind="ExternalInput").ap()
    mask_d = nc.dram_tensor("mask", [128, 128], BF16, kind="ExternalInput").ap()
    iden_d = nc.dram_tensor("iden", [128, 128], BF16, kind="ExternalInput").ap()
    out_d = nc.dram_tensor("out", [S, D_OUT], F32, kind="ExternalOutput").ap()

    Exp = mybir.ActivationFunctionType.Exp

    with contextlib.ExitStack() as ctx:
        const = ctx.enter_context(tc.tile_pool(name="const", bufs=1))
        persist = ctx.enter_context(tc.tile_pool(name="persist", bufs=1))
        small = ctx.enter_context(tc.tile_pool(name="small", bufs=4))
        outp = ctx.enter_context(tc.tile_pool(name="outp", bufs=4))

        # ---- constants ----
        # wq/wk arrive column-duplicated [512, 128] so the projection fills
        # both partition halves (enables PE row tiles T0+T8 in pass 1)
        wq_sb = const.tile([128, NCK * 128], BF16)
        wk_sb = const.tile([128, NCK * 128], BF16)
        wv_sb = const.tile([128, NCK * D_V], BF16)
        wo_sb = const.tile([D_V, D_OUT], BF16)
        mask_sb = const.tile([128, 128], BF16)
        iden_sb = const.tile([128, 128], BF16)
        for c in range(NCK):
            rows = slice(c * 128, (c + 1) * 128)
            nc.gpsimd.dma_start(out=wq_sb[:, c * 128:(c + 1) * 128], in_=wq_d[rows, :])
            nc.gpsimd.dma_start(out=wk_sb[:, c * 128:(c + 1) * 128], in_=wk_d[rows, :])
            nc.gpsimd.dma_start(out=wv_sb[:, c * D_V:(c + 1) * D_V], in_=wv_d[rows, :])
        nc.gpsimd.dma_start(out=wo_sb, in_=wo_d)
        nc.gpsimd.dma_start(out=mask_sb, in_=mask_d)
        nc.gpsimd.dma_start(out=iden_sb, in_=iden_d)

        # persistent activations
        qt = persist.tile([128, S], BF16)   # Q^T duplicated in both halves
        kt = persist.tile([128, S], BF16)   # K^T duplicated in both halves
        vpt = persist.tile([64, S], BF16)   # V^T
        vp = persist.tile([128, NJT * 65], BF16)  # V' tiles [128, 65] per jt

        # ones column of every V' tile: strided [128, NJT] memset
        nc.vector.memset(
            vp.rearrange("p (j w) -> p j w", w=65)[:, :, 64], 1.0)

        # ---- stage A: projections Q^T/K^T/V^T = W^T x^T ----
        with tc.tile_pool(name="xt", bufs=1) as xtp, \
             tc.tile_pool(name="psA", bufs=4, space="PSUM") as psA:
            xts = []
            for c in range(NCK):
                xt = xtp.tile([128, S], BF16, tag=f"xt{c}")
                nc.sync.dma_start(out=xt, in_=xT_d[c * 128:(c + 1) * 128, :])
                xts.append(xt)
            for w_sb, wid, dest, dcopy in (
                (wq_sb, 128, qt, nc.vector.tensor_copy),
                (wk_sb, 128, kt, nc.vector.tensor_copy),
                (wv_sb, 64, vpt, nc.vector.tensor_copy),
            ):
                for st in range(NCH):
                    sl = bass.ts(st, 512)
                    ps = psA.tile([wid, 512], F32, tag="psA",
                                  name=f"psA{st}_{wid}")
                    for c in range(NCK):
                        nc.tensor.matmul(
                            ps,
                            lhsT=w_sb[:, c * wid:(c + 1) * wid],
                            rhs=xts[c][:, sl],
                            start=(c == 0),
                            stop=(c == NCK - 1),
                        )
                    dcopy(dest[:, sl], ps)



        # ---- fused pass: S^T+exp, with O^T bursts filling PE exp-wait gaps ----
        from collections import deque
        pt_pool = ctx.enter_context(tc.tile_pool(name="pt", bufs=1))
        pts = []
        pending = deque()  # closures, each emits one PE-side step of pass 2

        def drain(n):
            if len(pending) > 48:
                n += 4
            for _ in range(n):
                if not pending:
                    return
                pending.popleft()()

        def vp_transpose(jt):
            def go():
                pst = psAcc.tile([128, 64], BF16, tag="bank", name=f"pst{jt}")
                nc.tensor.transpose(
                    pst,
                    vpt[:, jt * 128:(jt + 1) * 128],
                    iden_sb[0:64, 0:64],
                )
                nc.vector.tensor_copy(vp[:, jt * 65:jt * 65 + 64], pst)
            return go

        def enqueue_chunk(c, jt_last):
            acc = psAcc.tile([65, 512], F32, tag="bank", name=f"acc{c}")

            def ot_mm(j2):
                def go():
                    lo = max(c * 512, j2 * 128)
                    hi = (c + 1) * 512
                    nc.tensor.matmul(
                        acc[:, lo - c * 512:hi - c * 512],
                        lhsT=vp[:, j2 * 65:(j2 + 1) * 65],
                        rhs=pts[j2][:, lo - j2 * 128:hi - j2 * 128],
                        start=(j2 == 0),
                        stop=(j2 == jt_last),
                    )
                return go

            for j2 in range(jt_last + 1):
                pending.append(ot_mm(j2))

            def evac():
                ot_bf = small.tile([65, 512], BF16, tag="otbf")
                nc.vector.tensor_copy(ot_bf, acc)
                se_bf = small.tile([128, 4], BF16, tag="se_bf")
                for ib in range(4):
                    nc.gpsimd.dma_start(
                        out=se_bf[:, ib:ib + 1],
                        in_=ot_bf[64:65, ib * 128:(ib + 1) * 128],
                    ) if c < 4 else nc.sync.dma_start(
                        out=se_bf[:, ib:ib + 1],
                        in_=ot_bf[64:65, ib * 128:(ib + 1) * 128],
                    )
                rcols = small.tile([128, 4], F32, tag="rcols")
                nc.vector.reciprocal(rcols, se_bf)

                def out_proj(ib):
                    def go():
                        po = psAcc.tile([128, 512], F32, tag="bank",
                                        name=f"po{c}_{ib}")
                        nc.tensor.matmul(
                            po,
                            lhsT=ot_bf[0:64, ib * 128:(ib + 1) * 128],
                            rhs=wo_sb,
                            start=True,
                            stop=True,
                        )
                        ob = outp.tile([128, 512], F32, tag="ob")
                        if c >= 5:
                            nc.scalar.mul(ob, po, rcols[:, ib:ib + 1])
                        else:
                            nc.vector.tensor_scalar_mul(
                                ob, po, rcols[:, ib:ib + 1])
                        nc.sync.dma_start(
                            out=out_d[c * 512 + ib * 128:
                                      c * 512 + (ib + 1) * 128, :],
                            in_=ob,
                        )
                    return go

                for ib in range(4):
                    pending.append(out_proj(ib))

            pending.append(evac)

        with tc.tile_pool(name="psB", bufs=2, space="PSUM") as psB, \
             tc.tile_pool(name="psAcc", bufs=4, space="PSUM") as psAcc:
            for jt in range(NJT):
                pending.append(vp_transpose(jt))
            for jt in range(NJT):
                i0 = jt * 128           # diagonal start
                c0 = jt // 4            # first query chunk
                pt = pt_pool.tile([128, S - i0], BF16, tag=f"pt{jt}")
                pts.append(pt)
                # full 128-row contraction over the duplicated halves: each
                # product is summed twice (folded into exp scale), which keeps
                # the PE activity monitor warm (2.4 GHz) vs 64-row matmuls
                ktile = kt[:, jt * 128:(jt + 1) * 128]
                for g0 in range(c0, NCH, 2):
                    g1 = min(g0 + 2, NCH)
                    ps = psB.tile([128, 1024], F32, tag="psB")
                    for c in range(g0, g1):
                        lo = max(c * 512, i0)
                        hi = (c + 1) * 512
                        nc.tensor.matmul(
                            ps[:, (c - g0) * 512 + lo - c * 512:
                                  (c - g0) * 512 + hi - c * 512],
                            lhsT=ktile,
                            rhs=qt[:, lo:hi],
                            start=True,
                            stop=True,
                        )
                    glo = max(g0 * 512, i0)
                    ghi = g1 * 512
                    nc.scalar.activation(
                        pt[:, glo - i0:ghi - i0],
                        ps[:, glo - g0 * 512:ghi - g0 * 512],
                        Exp,
                        scale=0.0625,  # 1/sqrt(64) / 2 (duplicated contraction)
                    )
                    drain(3)  # O^T/out-proj work while ScalarE runs exp
                # causal mask on the diagonal 128x128 block
                nc.vector.tensor_mul(pt[:, 0:128], pt[:, 0:128], mask_sb)
                if jt % 4 == 3:
                    enqueue_chunk(jt // 4, jt)
            while pending:
                drain(8)


def _build():
    if "nc" in _CACHE:
        return _CACHE["nc"]
    nc = bacc.Bacc("TRN2", target_bir_lowering=False, debug=False)
    with tile.TileContext(nc) as tc:
        _emit(nc, tc, None)
    nc.compile()
    _CACHE["nc"] = nc
    return nc


def build_in_maps(x, W_q, W_k, W_v, W_o):
    bf = ml_dtypes.bfloat16
    xT = np.ascontiguousarray(x.reshape(S, D_IN).T).astype(bf)
    mask = np.triu(np.ones((128, 128), np.float32)).astype(bf)
    iden = np.eye(128, dtype=np.float32).astype(bf)
    in_maps = []
    for h in range(H):
        wq2 = np.concatenate([W_q[h], W_q[h]], axis=1)  # [512, 128]
        wk2 = np.concatenate([W_k[h], W_k[h]], axis=1)
        in_maps.append({
            "xT": xT,
            "wq": np.ascontiguousarray(wq2).astype(bf),
            "wk": np.ascontiguousarray(wk2).astype(bf),
            "wv": np.ascontiguousarray(W_v[h]).astype(bf),
            "wo": np.ascontiguousarray(W_o[h]).astype(bf),
            "mask": mask,
            "iden": iden,
        })
    return in_maps


def kernel(x, W_q, W_k, W_v, W_o):
    nc = _build()
    in_maps = build_in_maps(x, W_q, W_k, W_v, W_o)
    res = run_bass_kernel_spmd(nc, in_maps, core_ids=list(range(H)))
    out = np.zeros((S, D_OUT), np.float32)
    for h in range(H):
        out += res.results[h]["out"]
    return out[None]


# revision 37
# speedup vs baseline: 1.3940x; 1.0040x over previous
"""Multi-head causal attention on 8 TRN2 NeuronCores — one head per core.

Full inputs in, full output out. Per core (head h):
  Q^T/K^T/V^T = W^T x^T   (PE, bf16)
  S^T[j,i] = K_j . Q_i    (PE, causal-packed, flash-style)
  P^T = exp(S^T/8)        (ScalarE, no max-subtraction: |scores| << 1)
  O^T[v,i] accum += V'[j,(v|1)]^T P^T[j,i]  (PE; row 64 = sumexp)
  out[i,o] = (O^T[:,i]/sumexp_i)^T W_o      (PE + fused row scale on evac)
Host sums the 8 per-head partial outputs.
"""

import numpy as np
import ml_dtypes

import concourse.bass as bass
import concourse.mybir as mybir
import concourse.tile as tile
from concourse import bacc
from concourse.bass_utils import run_bass_kernel_spmd

BF16 = mybir.dt.bfloat16
F32 = mybir.dt.float32

S = 4096
D_IN = 512
D_K = 64
D_V = 64
D_OUT = 512
H = 8
NJT = S // 128   # 32 key tiles
NCH = S // 512   # 8 query chunks
NCK = D_IN // 128  # 4 contraction chunks for projections

_CACHE = {}


def _emit(nc, tc, ctx_pools):
    import contextlib

    xT_d = nc.dram_tensor("xT", [D_IN, S], BF16, kind="ExternalInput").ap()
    wq_d = nc.dram_tensor("wq", [D_IN, 128], BF16, kind="ExternalInput").ap()
    wk_d = nc.dram_tensor("wk", [D_IN, 128], BF16, kind="ExternalInput").ap()
    wv_d = nc.dram_tensor("wv", [D_IN, D_V], BF16, kind="ExternalInput").ap()
    wo_d = nc.dram_tensor("wo", [D_V, D_OUT], BF16, kind="ExternalInput").ap()
    mask_d = nc.dram_tensor("mask", [128, 128], BF16, kind="ExternalInput").ap()
    iden_d = nc.dram_tensor("iden", [128, 128], BF16, kind="ExternalInput").ap()
    out_d = nc.dram_tensor("out", [S, D_OUT], F32, kind="ExternalOutput").ap()

    Exp = mybir.ActivationFunctionType.Exp

    with contextlib.ExitStack() as ctx:
        const = ctx.enter_context(tc.tile_pool(name="const", bufs=1))
        persist = ctx.enter_context(tc.tile_pool(name="persist", bufs=1))
        small = ctx.enter_context(tc.tile_pool(name="small", bufs=4))
        outp = ctx.enter_context(tc.tile_pool(name="outp", bufs=4))

        # ---- constants ----
        # wq/wk arrive column-duplicated [512, 128] so the projection fills
        # both partition halves (enables PE row tiles T0+T8 in pass 1)
        wq_sb = const.tile([128, NCK * 128], BF16)
        wk_sb = const.tile([128, NCK * 128], BF16)
        wv_sb = const.tile([128, NCK * D_V], BF16)
        wo_sb = const.tile([D_V, D_OUT], BF16)
        mask_sb = const.tile([128, 128], BF16)
        iden_sb = const.tile([128, 128], BF16)
        for c in range(NCK):
            rows = slice(c * 128, (c + 1) * 128)
            nc.gpsimd.dma_start(out=wq_sb[:, c * 128:(c + 1) * 128], in_=wq_d[rows, :])
            nc.gpsimd.dma_start(out=wk_sb[:, c * 128:(c + 1) * 128], in_=wk_d[rows, :])
            nc.gpsimd.dma_start(out=wv_sb[:, c * D_V:(c + 1) * D_V], in_=wv_d[rows, :])
        nc.gpsimd.dma_start(out=wo_sb, in_=wo_d)
        nc.gpsimd.dma_start(out=mask_sb, in_=mask_d)
        nc.gpsimd.dma_start(out=iden_sb, in_=iden_d)

        # persistent activations
        qt = persist.tile([128, S], BF16)   # Q^T duplicated in both halves
        kt = persist.tile([128, S], BF16)   # K^T duplicated in both halves
        vpt = persist.tile([64, S], BF16)   # V^T
        vp = persist.tile([128, NJT * 65], BF16)  # V' tiles [128, 65] per jt

        # ones column of every V' tile: strided [128, NJT] memset
        nc.vector.memset(
            vp.rearrange("p (j w) -> p j w", w=65)[:, :, 64], 1.0)

        # ---- stage A: projections Q^T/K^T/V^T = W^T x^T ----
        with tc.tile_pool(name="xt", bufs=1) as xtp, \
             tc.tile_pool(name="psA", bufs=4, space="PSUM") as psA:
            xts = []
            for c in range(NCK):
                xt = xtp.tile([128, S], BF16, tag=f"xt{c}")
                nc.sync.dma_start(out=xt, in_=xT_d[c * 128:(c + 1) * 128, :])
                xts.append(xt)
            for w_sb, wid, dest, dcopy in (
                (wq_sb, 128, qt, nc.vector.tensor_copy),
                (wk_sb, 128, kt, nc.vector.tensor_copy),
                (wv_sb, 64, vpt, nc.vector.tensor_copy),
            ):
                for st in range(NCH):
                    sl = bass.ts(st, 512)
                    ps = psA.tile([wid, 512], F32, tag="psA",
                                  name=f"psA{st}_{wid}")
                    for c in range(NCK):
                        nc.tensor.matmul(
                            ps,
                            lhsT=w_sb[:, c * wid:(c + 1) * wid],
                            rhs=xts[c][:, sl],
                            start=(c == 0),
                            stop=(c == NCK - 1),
                        )
                    dcopy(dest[:, sl], ps)



        # ---- fused pass: S^T+exp, with O^T bursts filling PE exp-wait gaps ----
        from collections import deque
        pt_pool = ctx.enter_context(tc.tile_pool(name="pt", bufs=1))
        pts = []
        pending = deque()  # closures, each emits one PE-side step of pass 2

        def drain(n):
            if len(pending) > 48:
                n += 4
            for _ in range(n):
                if not pending:
                    return
                pending.popleft()()

        def vp_transpose(jt):
            def go():
                pst = psAcc.tile([128, 64], BF16, tag="bank", name=f"pst{jt}")
                nc.tensor.transpose(
                    pst,
                    vpt[:, jt * 128:(jt + 1) * 128],
                    iden_sb[0:64, 0:64],
                )
                nc.vector.tensor_copy(vp[:, jt * 65:jt * 65 + 64], pst)
            return go

        accs = {}

        def enqueue_ot(c, j2s):
            if c not in accs:
                accs[c] = psAcc.tile([65, 512], F32, tag="bank",
                                     name=f"acc{c}")
            acc = accs[c]
            jt_last = 4 * c + 3

            def ot_mm(j2):
                def go():
                    lo = max(c * 512, j2 * 128)
                    hi = (c + 1) * 512
                    nc.tensor.matmul(
                        acc[:, lo - c * 512:hi - c * 512],
                        lhsT=vp[:, j2 * 65:(j2 + 1) * 65],
                        rhs=pts[j2][:, lo - j2 * 128:hi - j2 * 128],
                        start=(j2 == 0),
                        stop=(j2 == jt_last),
                    )
                return go

            for j2 in j2s:
                pending.append(ot_mm(j2))

        def enqueue_fin(c):
            acc = accs[c]

            def evac():
                ot_bf = small.tile([65, 512], BF16, tag="otbf")
                nc.vector.tensor_copy(ot_bf, acc)
                se_bf = small.tile([128, 4], BF16, tag="se_bf")
                for ib in range(4):
                    nc.gpsimd.dma_start(
                        out=se_bf[:, ib:ib + 1],
                        in_=ot_bf[64:65, ib * 128:(ib + 1) * 128],
                    ) if c < 4 else nc.sync.dma_start(
                        out=se_bf[:, ib:ib + 1],
                        in_=ot_bf[64:65, ib * 128:(ib + 1) * 128],
                    )
                rcols = small.tile([128, 4], F32, tag="rcols")
                nc.vector.reciprocal(rcols, se_bf)

                def out_proj(ib):
                    def go():
                        po = psAcc.tile([128, 512], F32, tag="bank",
                                        name=f"po{c}_{ib}")
                        nc.tensor.matmul(
                            po,
                            lhsT=ot_bf[0:64, ib * 128:(ib + 1) * 128],
                            rhs=wo_sb,
                            start=True,
                            stop=True,
                        )
                        ob = outp.tile([128, 512], F32, tag="ob")
                        if c >= 5:
                            nc.scalar.mul(ob, po, rcols[:, ib:ib + 1])
                        else:
                            nc.vector.tensor_scalar_mul(
                                ob, po, rcols[:, ib:ib + 1])
                        nc.sync.dma_start(
                            out=out_d[c * 512 + ib * 128:
                                      c * 512 + (ib + 1) * 128, :],
                            in_=ob,
                        )
                    return go

                for ib in range(4):
                    pending.append(out_proj(ib))

            pending.append(evac)

        with tc.tile_pool(name="psB", bufs=2, space="PSUM") as psB, \
             tc.tile_pool(name="psAcc", bufs=4, space="PSUM") as psAcc:
            for jt in range(NJT):
                pending.append(vp_transpose(jt))
            for jt in range(NJT):
                i0 = jt * 128           # diagonal start
                c0 = jt // 4            # first query chunk
                pt = pt_pool.tile([128, S - i0], BF16, tag=f"pt{jt}")
                pts.append(pt)
                # full 128-row contraction over the duplicated halves: each
                # product is summed twice (folded into exp scale), which keeps
                # the PE activity monitor warm (2.4 GHz) vs 64-row matmuls
                ktile = kt[:, jt * 128:(jt + 1) * 128]
                for g0 in range(c0, NCH, 2):
                    g1 = min(g0 + 2, NCH)
                    ps = psB.tile([128, 1024], F32, tag="psB")
                    for c in range(g0, g1):
                        lo = max(c * 512, i0)
                        hi = (c + 1) * 512
                        nc.tensor.matmul(
                            ps[:, (c - g0) * 512 + lo - c * 512:
                                  (c - g0) * 512 + hi - c * 512],
                            lhsT=ktile,
                            rhs=qt[:, lo:hi],
                            start=True,
                            stop=True,
                        )
                    glo = max(g0 * 512, i0)
                    ghi = g1 * 512
                    nc.scalar.activation(
                        pt[:, glo - i0:ghi - i0],
                        ps[:, glo - g0 * 512:ghi - g0 * 512],
                        Exp,
                        scale=0.0625,  # 1/sqrt(64) / 2 (duplicated contraction)
                    )
                    drain(3)  # O^T/out-proj work while ScalarE runs exp
                # causal mask on the diagonal 128x128 block
                nc.vector.tensor_mul(pt[:, 0:128], pt[:, 0:128], mask_sb)
                if jt % 4 == 1:
                    # chunk c=jt//4: most contributions are already available
                    enqueue_ot(jt // 4, range(0, jt + 1))
                elif jt % 4 == 3:
                    c = jt // 4
                    enqueue_ot(c, range(jt - 1, jt + 1))
                    enqueue_fin(c)
            while pending:
                drain(8)


def _build():
    if "nc" in _CACHE:
        return _CACHE["nc"]
    nc = bacc.Bacc("TRN2", target_bir_lowering=False, debug=False)
    with tile.TileContext(nc) as tc:
        _emit(nc, tc, None)
    nc.compile()
    _CACHE["nc"] = nc
    return nc


def build_in_maps(x, W_q, W_k, W_v, W_o):
    bf = ml_dtypes.bfloat16
    xT = np.ascontiguousarray(x.reshape(S, D_IN).T).astype(bf)
    mask = np.triu(np.ones((128, 128), np.float32)).astype(bf)
    iden = np.eye(128, dtype=np.float32).astype(bf)
    in_maps = []
    for h in range(H):
        wq2 = np.concatenate([W_q[h], W_q[h]], axis=1)  # [512, 128]
        wk2 = np.concatenate([W_k[h], W_k[h]], axis=1)
        in_maps.append({
            "xT": xT,
            "wq": np.ascontiguousarray(wq2).astype(bf),
            "wk": np.ascontiguousarray(wk2).astype(bf),
            "wv": np.ascontiguousarray(W_v[h]).astype(bf),
            "wo": np.ascontiguousarray(W_o[h]).astype(bf),
            "mask": mask,
            "iden": iden,
        })
    return in_maps


def kernel(x, W_q, W_k, W_v, W_o):
    nc = _build()
    in_maps = build_in_maps(x, W_q, W_k, W_v, W_o)
    res = run_bass_kernel_spmd(nc, in_maps, core_ids=list(range(H)))
    out = np.zeros((S, D_OUT), np.float32)
    for h in range(H):
        out += res.results[h]["out"]
    return out[None]


# revision 47
# speedup vs baseline: 1.4656x; 1.0514x over previous
"""Multi-head causal attention on 8 TRN2 NeuronCores — one head per core.

Full inputs in, full output out. Per core (head h):
  Q^T/K^T/V^T = W^T x^T   (PE, bf16)
  S^T[j,i] = K_j . Q_i    (PE, causal-packed, flash-style)
  P^T = exp(S^T/8)        (ScalarE, no max-subtraction: |scores| << 1)
  O^T[v,i] accum += V'[j,(v|1)]^T P^T[j,i]  (PE; row 64 = sumexp)
  out[i,o] = (O^T[:,i]/sumexp_i)^T W_o      (PE + fused row scale on evac)
Host sums the 8 per-head partial outputs.
"""

import numpy as np
import ml_dtypes

import concourse.bass as bass
import concourse.mybir as mybir
import concourse.tile as tile
from concourse import bacc
from concourse.bass_utils import run_bass_kernel_spmd

BF16 = mybir.dt.bfloat16
F32 = mybir.dt.float32

S = 4096
D_IN = 512
D_K = 64
D_V = 64
D_OUT = 512
H = 8
NJT = S // 128   # 32 key tiles
NCH = S // 512   # 8 query chunks
NCK = D_IN // 128  # 4 contraction chunks for projections

_CACHE = {}


def _emit(nc, tc, ctx_pools):
    import contextlib

    xT_d = nc.dram_tensor("xT", [D_IN, S], BF16, kind="ExternalInput").ap()
    wq_d = nc.dram_tensor("wq", [D_IN, 128], BF16, kind="ExternalInput").ap()
    wk_d = nc.dram_tensor("wk", [D_IN, 128], BF16, kind="ExternalInput").ap()
    wv_d = nc.dram_tensor("wv", [D_IN, D_V], BF16, kind="ExternalInput").ap()
    wo_d = nc.dram_tensor("wo", [D_V, D_OUT], BF16, kind="ExternalInput").ap()
    mask_d = nc.dram_tensor("mask", [128, 128], BF16, kind="ExternalInput").ap()
    iden_d = nc.dram_tensor("iden", [128, 128], BF16, kind="ExternalInput").ap()
    out_d = nc.dram_tensor("out", [S, D_OUT], F32, kind="ExternalOutput").ap()

    Exp = mybir.ActivationFunctionType.Exp

    with contextlib.ExitStack() as ctx:
        const = ctx.enter_context(tc.tile_pool(name="const", bufs=1))
        persist = ctx.enter_context(tc.tile_pool(name="persist", bufs=1))
        small = ctx.enter_context(tc.tile_pool(name="small", bufs=3))
        outp = ctx.enter_context(tc.tile_pool(name="outp", bufs=3))

        # ---- constants ----
        # wq/wk arrive column-duplicated [512, 128] so the projection fills
        # both partition halves (enables PE row tiles T0+T8 in pass 1)
        wq_sb = const.tile([128, NCK * 128], BF16)
        wk_sb = const.tile([128, NCK * 128], BF16)
        wv_sb = const.tile([128, NCK * D_V], BF16)
        wo_sb = const.tile([D_V, D_OUT], BF16)
        mask_sb = const.tile([128, 128], BF16)
        iden_sb = const.tile([128, 128], BF16)
        for c in range(NCK):
            rows = slice(c * 128, (c + 1) * 128)
            nc.gpsimd.dma_start(out=wq_sb[:, c * 128:(c + 1) * 128], in_=wq_d[rows, :])
            nc.gpsimd.dma_start(out=wk_sb[:, c * 128:(c + 1) * 128], in_=wk_d[rows, :])
            nc.gpsimd.dma_start(out=wv_sb[:, c * D_V:(c + 1) * D_V], in_=wv_d[rows, :])
        nc.gpsimd.dma_start(out=wo_sb, in_=wo_d)
        nc.gpsimd.dma_start(out=mask_sb, in_=mask_d)
        nc.gpsimd.dma_start(out=iden_sb, in_=iden_d)

        # persistent activations
        qt = persist.tile([128, S], BF16)   # Q^T duplicated in both halves
        kt = persist.tile([128, S], BF16)   # K^T duplicated in both halves
        vp = persist.tile([128, NJT * 65], BF16)  # V' tiles [128, 65] per jt

        # ones column of every V' tile: strided [128, NJT] memset
        nc.vector.memset(
            vp.rearrange("p (j w) -> p j w", w=65)[:, :, 64], 1.0)

        # ---- stage A: only the projections the first S^T needs (all of Q,
        # K s-tile 0) run eagerly; the rest become pass-1 PE filler work ----
        pt_pool = ctx.enter_context(tc.tile_pool(name="pt", bufs=1))
        xtp_ctx = contextlib.ExitStack()
        xtp = xtp_ctx.enter_context(tc.tile_pool(name="xt", bufs=1))
        vpt = xtp.tile([64, S], BF16)       # V^T (dies with x^T tiles)
        xts = []
        for c in range(NCK):
            xt = xtp.tile([128, S], BF16, tag=f"xt{c}")
            nc.sync.dma_start(out=xt, in_=xT_d[c * 128:(c + 1) * 128, :])
            xts.append(xt)

        def proj(w_sb, wid, dest, dcopy, st, pool, tag="psA"):
            sl = bass.ts(st, 512)
            ps = pool.tile([wid, 512], F32, tag=tag,
                           name=f"ps{wid}_{st}")
            for c in range(NCK):
                nc.tensor.matmul(
                    ps,
                    lhsT=w_sb[:, c * wid:(c + 1) * wid],
                    rhs=xts[c][:, sl],
                    start=(c == 0),
                    stop=(c == NCK - 1),
                )
            dcopy(dest[:, sl], ps)

        with tc.tile_pool(name="psA", bufs=4, space="PSUM") as psA:
            for st in range(NCH):
                proj(wq_sb, 128, qt, nc.vector.tensor_copy, st, psA)
            proj(wk_sb, 128, kt, nc.vector.tensor_copy, 0, psA)



        # ---- fused pass: S^T+exp, with O^T bursts filling PE exp-wait gaps ----
        from collections import deque
        pts = []
        pending = deque()  # closures, each emits one PE-side step of pass 2

        def drain(n):
            if len(pending) > 48:
                n += 4
            for _ in range(n):
                if not pending:
                    return
                pending.popleft()()

        def vp_transpose(jt):
            def go():
                pst = psAcc.tile([128, 64], BF16, tag="bank", name=f"pst{jt}")
                nc.tensor.transpose(
                    pst,
                    vpt[:, jt * 128:(jt + 1) * 128],
                    iden_sb[0:64, 0:64],
                )
                nc.vector.tensor_copy(vp[:, jt * 65:jt * 65 + 64], pst)
            return go

        def filler_proj(w_sb, wid, dest, st):
            def go():
                proj(w_sb, wid, dest, nc.vector.tensor_copy, st, psAcc,
                     tag="bank")
            return go

        accs = {}

        def enqueue_ot(c, j2s):
            if c not in accs:
                accs[c] = psAcc.tile([65, 512], F32, tag="bank",
                                     name=f"acc{c}")
            acc = accs[c]
            jt_last = 4 * c + 3

            def ot_mm(j2):
                def go():
                    lo = max(c * 512, j2 * 128)
                    hi = (c + 1) * 512
                    nc.tensor.matmul(
                        acc[:, lo - c * 512:hi - c * 512],
                        lhsT=vp[:, j2 * 65:(j2 + 1) * 65],
                        rhs=pts[j2][:, lo - j2 * 128:hi - j2 * 128],
                        start=(j2 == 0),
                        stop=(j2 == jt_last),
                    )
                return go

            for j2 in j2s:
                pending.append(ot_mm(j2))

        def enqueue_fin(c):
            acc = accs[c]

            def evac():
                ot_bf = small.tile([65, 512], BF16, tag="otbf")
                nc.vector.tensor_copy(ot_bf, acc)
                se_bf = small.tile([128, 4], BF16, tag="se_bf")
                for ib in range(4):
                    nc.gpsimd.dma_start(
                        out=se_bf[:, ib:ib + 1],
                        in_=ot_bf[64:65, ib * 128:(ib + 1) * 128],
                    ) if c < 4 else nc.sync.dma_start(
                        out=se_bf[:, ib:ib + 1],
                        in_=ot_bf[64:65, ib * 128:(ib + 1) * 128],
                    )
                rcols = small.tile([128, 4], F32, tag="rcols")
                nc.vector.reciprocal(rcols, se_bf)

                def out_proj(ib):
                    def go():
                        po = psAcc.tile([128, 512], F32, tag="bank",
                                        name=f"po{c}_{ib}")
                        nc.tensor.matmul(
                            po,
                            lhsT=ot_bf[0:64, ib * 128:(ib + 1) * 128],
                            rhs=wo_sb,
                            start=True,
                            stop=True,
                        )
                        ob = outp.tile([128, 512], F32, tag="ob")
                        if c >= 5:
                            nc.scalar.mul(ob, po, rcols[:, ib:ib + 1])
                        else:
                            nc.vector.tensor_scalar_mul(
                                ob, po, rcols[:, ib:ib + 1])
                        nc.sync.dma_start(
                            out=out_d[c * 512 + ib * 128:
                                      c * 512 + (ib + 1) * 128, :],
                            in_=ob,
                        )
                    return go

                for ib in range(4):
                    pending.append(out_proj(ib))

            pending.append(evac)

        with tc.tile_pool(name="psB", bufs=2, space="PSUM") as psB, \
             tc.tile_pool(name="psAcc", bufs=4, space="PSUM") as psAcc:
            # filler work: remaining K projections, V projections + V' tiles
            for st in range(1, NCH):
                pending.append(filler_proj(wk_sb, 128, kt, st))
            for st in range(NCH):
                pending.append(filler_proj(wv_sb, 64, vpt, st))
                for j2 in range(4 * st, 4 * st + 4):
                    pending.append(vp_transpose(j2))
            for jt in range(NJT):
                i0 = jt * 128           # diagonal start
                c0 = jt // 4            # first query chunk
                pt = pt_pool.tile([128, S - i0], BF16, tag=f"pt{jt}")
                pts.append(pt)
                # full 128-row contraction over the duplicated halves: each
                # product is summed twice (folded into exp scale), which keeps
                # the PE activity monitor warm (2.4 GHz) vs 64-row matmuls
                ktile = kt[:, jt * 128:(jt + 1) * 128]
                for g0 in range(c0, NCH, 2):
                    g1 = min(g0 + 2, NCH)
                    ps = psB.tile([128, 1024], F32, tag="psB")
                    for c in range(g0, g1):
                        lo = max(c * 512, i0)
                        hi = (c + 1) * 512
                        nc.tensor.matmul(
                            ps[:, (c - g0) * 512 + lo - c * 512:
                                  (c - g0) * 512 + hi - c * 512],
                            lhsT=ktile,
                            rhs=qt[:, lo:hi],
                            start=True,
                            stop=True,
                        )
                    glo = max(g0 * 512, i0)
                    ghi = g1 * 512
                    nc.scalar.activation(
                        pt[:, glo - i0:ghi - i0],
                        ps[:, glo - g0 * 512:ghi - g0 * 512],
                        Exp,
                        scale=0.0625,  # 1/sqrt(64) / 2 (duplicated contraction)
                    )
                    drain(3)  # O^T/out-proj work while ScalarE runs exp
                # causal mask on the diagonal 128x128 block
                nc.vector.tensor_mul(pt[:, 0:128], pt[:, 0:128], mask_sb)
                if jt % 4 == 1:
                    # chunk c=jt//4: most contributions are already available
                    enqueue_ot(jt // 4, range(0, jt + 1))
                elif jt % 4 == 3:
                    c = jt // 4
                    enqueue_ot(c, range(jt - 1, jt + 1))
                    enqueue_fin(c)
                if jt == 9:
                    # force-drain any remaining fillers, then free the x^T
                    # tiles before the P^T pool reaches peak size
                    while pending:
                        pending.popleft()()
                    xtp_ctx.close()
            while pending:
                drain(8)


def _build():
    if "nc" in _CACHE:
        return _CACHE["nc"]
    nc = bacc.Bacc("TRN2", target_bir_lowering=False, debug=False)
    with tile.TileContext(nc) as tc:
        _emit(nc, tc, None)
    nc.compile()
    _CACHE["nc"] = nc
    return nc


def build_in_maps(x, W_q, W_k, W_v, W_o):
    bf = ml_dtypes.bfloat16
    xT = np.ascontiguousarray(x.reshape(S, D_IN).T).astype(bf)
    mask = np.triu(np.ones((128, 128), np.float32)).astype(bf)
    iden = np.eye(128, dtype=np.float32).astype(bf)
    in_maps = []
    for h in range(H):
        wq2 = np.concatenate([W_q[h], W_q[h]], axis=1)  # [512, 128]
        wk2 = np.concatenate([W_k[h], W_k[h]], axis=1)
        in_maps.append({
            "xT": xT,
            "wq": np.ascontiguousarray(wq2).astype(bf),
            "wk": np.ascontiguousarray(wk2).astype(bf),
            "wv": np.ascontiguousarray(W_v[h]).astype(bf),
            "wo": np.ascontiguousarray(W_o[h]).astype(bf),
            "mask": mask,
            "iden": iden,
        })
    return in_maps


def kernel(x, W_q, W_k, W_v, W_o):
    nc = _build()
    in_maps = build_in_maps(x, W_q, W_k, W_v, W_o)
    res = run_bass_kernel_spmd(nc, in_maps, core_ids=list(range(H)))
    out = np.zeros((S, D_OUT), np.float32)
    for h in range(H):
        out += res.results[h]["out"]
    return out[None]


# revision 49
# speedup vs baseline: 1.4766x; 1.0075x over previous
"""Multi-head causal attention on 8 TRN2 NeuronCores — one head per core.

Full inputs in, full output out. Per core (head h):
  Q^T/K^T/V^T = W^T x^T   (PE, bf16)
  S^T[j,i] = K_j . Q_i    (PE, causal-packed, flash-style)
  P^T = exp(S^T/8)        (ScalarE, no max-subtraction: |scores| << 1)
  O^T[v,i] accum += V'[j,(v|1)]^T P^T[j,i]  (PE; row 64 = sumexp)
  out[i,o] = (O^T[:,i]/sumexp_i)^T W_o      (PE + fused row scale on evac)
Host sums the 8 per-head partial outputs.
"""

import numpy as np
import ml_dtypes

import concourse.bass as bass
import concourse.mybir as mybir
import concourse.tile as tile
from concourse import bacc
from concourse.bass_utils import run_bass_kernel_spmd

BF16 = mybir.dt.bfloat16
F32 = mybir.dt.float32

S = 4096
D_IN = 512
D_K = 64
D_V = 64
D_OUT = 512
H = 8
NJT = S // 128   # 32 key tiles
NCH = S // 512   # 8 query chunks
NCK = D_IN // 128  # 4 contraction chunks for projections

_CACHE = {}


def _emit(nc, tc, ctx_pools):
    import contextlib

    xT_d = nc.dram_tensor("xT", [D_IN, S], BF16, kind="ExternalInput").ap()
    wq_d = nc.dram_tensor("wq", [D_IN, 128], BF16, kind="ExternalInput").ap()
    wk_d = nc.dram_tensor("wk", [D_IN, 128], BF16, kind="ExternalInput").ap()
    wv_d = nc.dram_tensor("wv", [D_IN, D_V], BF16, kind="ExternalInput").ap()
    wo_d = nc.dram_tensor("wo", [D_V, D_OUT], BF16, kind="ExternalInput").ap()
    mask_d = nc.dram_tensor("mask", [128, 128], BF16, kind="ExternalInput").ap()
    iden_d = nc.dram_tensor("iden", [128, 128], BF16, kind="ExternalInput").ap()
    out_d = nc.dram_tensor("out", [S, D_OUT], F32, kind="ExternalOutput").ap()

    Exp = mybir.ActivationFunctionType.Exp

    with contextlib.ExitStack() as ctx:
        const = ctx.enter_context(tc.tile_pool(name="const", bufs=1))
        persist = ctx.enter_context(tc.tile_pool(name="persist", bufs=1))
        small = ctx.enter_context(tc.tile_pool(name="small", bufs=3))
        outp = ctx.enter_context(tc.tile_pool(name="outp", bufs=3))

        # ---- constants ----
        # wq/wk arrive column-duplicated [512, 128] so the projection fills
        # both partition halves (enables PE row tiles T0+T8 in pass 1)
        wq_sb = const.tile([128, NCK * 128], BF16)
        wk_sb = const.tile([128, NCK * 128], BF16)
        wv_sb = const.tile([128, NCK * D_V], BF16)
        wo_sb = const.tile([D_V, D_OUT], BF16)
        mask_sb = const.tile([128, 128], BF16)
        iden_sb = const.tile([128, 128], BF16)
        for c in range(NCK):
            rows = slice(c * 128, (c + 1) * 128)
            nc.gpsimd.dma_start(out=wq_sb[:, c * 128:(c + 1) * 128], in_=wq_d[rows, :])
            nc.gpsimd.dma_start(out=wk_sb[:, c * 128:(c + 1) * 128], in_=wk_d[rows, :])
            nc.gpsimd.dma_start(out=wv_sb[:, c * D_V:(c + 1) * D_V], in_=wv_d[rows, :])
        nc.gpsimd.dma_start(out=wo_sb, in_=wo_d)
        nc.gpsimd.dma_start(out=mask_sb, in_=mask_d)
        nc.gpsimd.dma_start(out=iden_sb, in_=iden_d)

        # persistent activations
        qt = persist.tile([128, S], BF16)   # Q^T duplicated in both halves
        kt = persist.tile([128, S], BF16)   # K^T duplicated in both halves
        vp = persist.tile([128, NJT * 65], BF16)  # V' tiles [128, 65] per jt

        # ones column of every V' tile: strided [128, NJT] memset
        nc.vector.memset(
            vp.rearrange("p (j w) -> p j w", w=65)[:, :, 64], 1.0)

        # ---- stage A: only the projections the first S^T needs (all of Q,
        # K s-tile 0) run eagerly; the rest become pass-1 PE filler work ----
        pt_pool = ctx.enter_context(tc.tile_pool(name="pt", bufs=1))
        xtp_ctx = contextlib.ExitStack()
        xtp = xtp_ctx.enter_context(tc.tile_pool(name="xt", bufs=1))
        vpt = xtp.tile([64, S], BF16)       # V^T (dies with x^T tiles)
        xts = []
        for c in range(NCK):
            xt = xtp.tile([128, S], BF16, tag=f"xt{c}")
            nc.sync.dma_start(out=xt, in_=xT_d[c * 128:(c + 1) * 128, :])
            xts.append(xt)

        def proj(w_sb, wid, dest, dcopy, st, pool, tag="psA"):
            sl = bass.ts(st, 512)
            ps = pool.tile([wid, 512], F32, tag=tag,
                           name=f"ps{wid}_{st}")
            for c in range(NCK):
                nc.tensor.matmul(
                    ps,
                    lhsT=w_sb[:, c * wid:(c + 1) * wid],
                    rhs=xts[c][:, sl],
                    start=(c == 0),
                    stop=(c == NCK - 1),
                )
            dcopy(dest[:, sl], ps)

        with tc.tile_pool(name="psA", bufs=4, space="PSUM") as psA:
            for st in range(2):
                proj(wq_sb, 128, qt, nc.vector.tensor_copy, st, psA)
            proj(wk_sb, 128, kt, nc.vector.tensor_copy, 0, psA)



        # ---- fused pass: S^T+exp, with O^T bursts filling PE exp-wait gaps ----
        from collections import deque
        pts = []
        pending = deque()  # closures, each emits one PE-side step of pass 2

        def drain(n):
            if len(pending) > 48:
                n += 4
            for _ in range(n):
                if not pending:
                    return
                pending.popleft()()

        def vp_transpose(jt):
            def go():
                pst = psAcc.tile([128, 64], BF16, tag="bank", name=f"pst{jt}")
                nc.tensor.transpose(
                    pst,
                    vpt[:, jt * 128:(jt + 1) * 128],
                    iden_sb[0:64, 0:64],
                )
                nc.vector.tensor_copy(vp[:, jt * 65:jt * 65 + 64], pst)
            return go

        def filler_proj(w_sb, wid, dest, st):
            def go():
                proj(w_sb, wid, dest, nc.vector.tensor_copy, st, psAcc,
                     tag="bank")
            return go

        accs = {}

        def enqueue_ot(c, j2s):
            if c not in accs:
                accs[c] = psAcc.tile([65, 512], F32, tag="bank",
                                     name=f"acc{c}")
            acc = accs[c]
            jt_last = 4 * c + 3

            def ot_mm(j2):
                def go():
                    lo = max(c * 512, j2 * 128)
                    hi = (c + 1) * 512
                    nc.tensor.matmul(
                        acc[:, lo - c * 512:hi - c * 512],
                        lhsT=vp[:, j2 * 65:(j2 + 1) * 65],
                        rhs=pts[j2][:, lo - j2 * 128:hi - j2 * 128],
                        start=(j2 == 0),
                        stop=(j2 == jt_last),
                    )
                return go

            for j2 in j2s:
                pending.append(ot_mm(j2))

        def enqueue_fin(c):
            acc = accs[c]

            def evac():
                ot_bf = small.tile([65, 512], BF16, tag="otbf")
                nc.vector.tensor_copy(ot_bf, acc)
                se_bf = small.tile([128, 4], BF16, tag="se_bf")
                for ib in range(4):
                    nc.gpsimd.dma_start(
                        out=se_bf[:, ib:ib + 1],
                        in_=ot_bf[64:65, ib * 128:(ib + 1) * 128],
                    ) if c < 4 else nc.sync.dma_start(
                        out=se_bf[:, ib:ib + 1],
                        in_=ot_bf[64:65, ib * 128:(ib + 1) * 128],
                    )
                rcols = small.tile([128, 4], F32, tag="rcols")
                nc.vector.reciprocal(rcols, se_bf)

                def out_proj(ib):
                    def go():
                        po = psAcc.tile([128, 512], F32, tag="bank",
                                        name=f"po{c}_{ib}")
                        nc.tensor.matmul(
                            po,
                            lhsT=ot_bf[0:64, ib * 128:(ib + 1) * 128],
                            rhs=wo_sb,
                            start=True,
                            stop=True,
                        )
                        ob = outp.tile([128, 512], F32, tag="ob")
                        if c >= 5:
                            nc.scalar.mul(ob, po, rcols[:, ib:ib + 1])
                        else:
                            nc.vector.tensor_scalar_mul(
                                ob, po, rcols[:, ib:ib + 1])
                        nc.sync.dma_start(
                            out=out_d[c * 512 + ib * 128:
                                      c * 512 + (ib + 1) * 128, :],
                            in_=ob,
                        )
                    return go

                for ib in range(4):
                    pending.append(out_proj(ib))

            pending.append(evac)

        with tc.tile_pool(name="psB", bufs=2, space="PSUM") as psB, \
             tc.tile_pool(name="psAcc", bufs=4, space="PSUM") as psAcc:
            # filler work: remaining Q/K projections, V projections + V' tiles
            for st in range(2, NCH):
                pending.append(filler_proj(wq_sb, 128, qt, st))
            for st in range(1, NCH):
                pending.append(filler_proj(wk_sb, 128, kt, st))
            for st in range(NCH):
                pending.append(filler_proj(wv_sb, 64, vpt, st))
                for j2 in range(4 * st, 4 * st + 4):
                    pending.append(vp_transpose(j2))
            for jt in range(NJT):
                i0 = jt * 128           # diagonal start
                c0 = jt // 4            # first query chunk
                pt = pt_pool.tile([128, S - i0], BF16, tag=f"pt{jt}")
                pts.append(pt)
                # full 128-row contraction over the duplicated halves: each
                # product is summed twice (folded into exp scale), which keeps
                # the PE activity monitor warm (2.4 GHz) vs 64-row matmuls
                ktile = kt[:, jt * 128:(jt + 1) * 128]
                for g0 in range(c0, NCH, 2):
                    g1 = min(g0 + 2, NCH)
                    ps = psB.tile([128, 1024], F32, tag="psB")
                    for c in range(g0, g1):
                        lo = max(c * 512, i0)
                        hi = (c + 1) * 512
                        nc.tensor.matmul(
                            ps[:, (c - g0) * 512 + lo - c * 512:
                                  (c - g0) * 512 + hi - c * 512],
                            lhsT=ktile,
                            rhs=qt[:, lo:hi],
                            start=True,
                            stop=True,
                        )
                    glo = max(g0 * 512, i0)
                    ghi = g1 * 512
                    nc.scalar.activation(
                        pt[:, glo - i0:ghi - i0],
                        ps[:, glo - g0 * 512:ghi - g0 * 512],
                        Exp,
                        scale=0.0625,  # 1/sqrt(64) / 2 (duplicated contraction)
                    )
                    drain(3)  # O^T/out-proj work while ScalarE runs exp
                # causal mask on the diagonal 128x128 block
                nc.vector.tensor_mul(pt[:, 0:128], pt[:, 0:128], mask_sb)
                if jt % 4 == 1:
                    # chunk c=jt//4: most contributions are already available
                    enqueue_ot(jt // 4, range(0, jt + 1))
                elif jt % 4 == 3:
                    c = jt // 4
                    enqueue_ot(c, range(jt - 1, jt + 1))
                    enqueue_fin(c)
                if jt == 9:
                    # force-drain any remaining fillers, then free the x^T
                    # tiles before the P^T pool reaches peak size
                    while pending:
                        pending.popleft()()
                    xtp_ctx.close()
            while pending:
                drain(8)


def _build():
    if "nc" in _CACHE:
        return _CACHE["nc"]
    nc = bacc.Bacc("TRN2", target_bir_lowering=False, debug=False)
    with tile.TileContext(nc) as tc:
        _emit(nc, tc, None)
    nc.compile()
    _CACHE["nc"] = nc
    return nc


def build_in_maps(x, W_q, W_k, W_v, W_o):
    bf = ml_dtypes.bfloat16
    xT = np.ascontiguousarray(x.reshape(S, D_IN).T).astype(bf)
    mask = np.triu(np.ones((128, 128), np.float32)).astype(bf)
    iden = np.eye(128, dtype=np.float32).astype(bf)
    in_maps = []
    for h in range(H):
        wq2 = np.concatenate([W_q[h], W_q[h]], axis=1)  # [512, 128]
        wk2 = np.concatenate([W_k[h], W_k[h]], axis=1)
        in_maps.append({
            "xT": xT,
            "wq": np.ascontiguousarray(wq2).astype(bf),
            "wk": np.ascontiguousarray(wk2).astype(bf),
            "wv": np.ascontiguousarray(W_v[h]).astype(bf),
            "wo": np.ascontiguousarray(W_o[h]).astype(bf),
            "mask": mask,
            "iden": iden,
        })
    return in_maps


def kernel(x, W_q, W_k, W_v, W_o):
    nc = _build()
    in_maps = build_in_maps(x, W_q, W_k, W_v, W_o)
    res = run_bass_kernel_spmd(nc, in_maps, core_ids=list(range(H)))
    out = np.zeros((S, D_OUT), np.float32)
    for h in range(H):
        out += res.results[h]["out"]
    return out[None]
